# revision 1
# baseline (speedup 1.0000x reference)
"""Local causal (sliding-window) attention block on 8 TRN2 NeuronCores.

Reference computation (per batch b):
    h = LayerNorm(x) * gamma + beta
    Q = h@Wq, K = h@Wk, V = h@Wv          (heads: 16 x 64)
    S = QK^T/sqrt(dk) masked to causal band of width 256
    out = x + softmax(S)@V @ Wo + bo

Sharding: 8 cores = 2 batches x 4 head-groups (4 heads each).
Each core computes LN(x_b), its head-group's Q/K/V, banded attention,
and a partial out-projection  attn_g @ Wo[g]  (token-major, [T, D]).
Host reduces: out[b] = x[b] + sum_g partial[b,g] + bo.

gamma (and 1/sqrt(dk) for Q) are folded into the projection weights on
the host; beta enters via folded bias rows beta@W.
"""

import os

import numpy as np

import concourse.bass as bass
import concourse.tile as tile
from concourse import bacc, mybir
from concourse.bass_utils import run_bass_kernel_spmd

F32 = mybir.dt.float32
F32R = mybir.dt.float32r
BF16 = mybir.dt.bfloat16

T = 2048          # tokens per batch
D = 1024          # model dim
HG = 4            # heads per core
DK = 64           # head dim
DG = HG * DK      # head-group feature width (256)
WIN = 256         # attention window
P = 128           # partitions
NT = T // P       # 16 token tiles
KC = D // P       # 8 feature chunks
LN_EPS = 1e-5
MASKVAL = -1e9

# filled by test.py via run(trace=True)
LAST_PROFILE = {}


def _rc(ap):
    return ap


def _body(tc):
    nc = tc.nc

    x = nc.dram_tensor("x", [T, D], F32, kind="ExternalInput").ap()
    wq = nc.dram_tensor("wq", [D, DG], F32R, kind="ExternalInput").ap()
    wk = nc.dram_tensor("wk", [D, DG], F32R, kind="ExternalInput").ap()
    wv = nc.dram_tensor("wv", [D, DG], F32R, kind="ExternalInput").ap()
    wo = nc.dram_tensor("wo", [DG, D], F32R, kind="ExternalInput").ap()
    bq = nc.dram_tensor("bq", [P, DG // P], F32, kind="ExternalInput").ap()
    bk = nc.dram_tensor("bk", [P, DG // P], F32, kind="ExternalInput").ap()
    bv = nc.dram_tensor("bv", [P, DG], F32, kind="ExternalInput").ap()
    mup = nc.dram_tensor("mup", [P, P], F32, kind="ExternalInput").ap()
    mlo = nc.dram_tensor("mlo", [P, P], F32, kind="ExternalInput").ap()
    idf = nc.dram_tensor("idf", [P, P], F32R, kind="ExternalInput").ap()
    idb = nc.dram_tensor("idb", [P, P], BF16, kind="ExternalInput").ap()
    partial = nc.dram_tensor("partial", [T, D], F32, kind="ExternalOutput").ap()

    with (
        tc.tile_pool(name="consts", bufs=1) as consts,
        tc.tile_pool(name="big", bufs=1) as big,
    ):
        # ---- resident SBUF tensors ----
        wq_sb = consts.tile([P, KC, DG], F32R, tag="wq")
        wk_sb = consts.tile([P, KC, DG], F32R, tag="wk")
        wv_sb = consts.tile([P, KC, DG], F32R, tag="wv")
        wo_sb = consts.tile([P, DG // P, D], F32R, tag="wo")
        bq_sb = consts.tile([P, DG // P], F32, tag="bq")
        bk_sb = consts.tile([P, DG // P], F32, tag="bk")
        bv_sb = consts.tile([P, DG], F32, tag="bv")
        mup_sb = consts.tile([P, P], F32, tag="mup")
        mlo_sb = consts.tile([P, P], F32, tag="mlo")
        idf_sb = consts.tile([P, P], F32R, tag="idf")
        idb_sb = consts.tile([P, P], BF16, tag="idb")
        eps_sb = consts.tile([P, 1], F32, tag="eps")

        nc.sync.dma_start(out=wq_sb, in_=wq.rearrange("(c p) n -> p c n", p=P))
        nc.sync.dma_start(out=wk_sb, in_=wk.rearrange("(c p) n -> p c n", p=P))
        nc.sync.dma_start(out=wv_sb, in_=wv.rearrange("(c p) n -> p c n", p=P))
        nc.sync.dma_start(out=wo_sb, in_=wo.rearrange("(c p) n -> p c n", p=P))
        nc.sync.dma_start(out=bq_sb, in_=bq)
        nc.sync.dma_start(out=bk_sb, in_=bk)
        nc.sync.dma_start(out=bv_sb, in_=bv)
        nc.sync.dma_start(out=mup_sb, in_=mup)
        nc.sync.dma_start(out=mlo_sb, in_=mlo)
        nc.sync.dma_start(out=idf_sb, in_=idf)
        nc.sync.dma_start(out=idb_sb, in_=idb)
        nc.vector.memset(eps_sb, LN_EPS)

        # h^T (LayerNormed x, feature-major), Q^T/K^T (feature-major),
        # V (token-major, bf16), O^T (attention output, feature-major)
        ht_sb = big.tile([P, KC, T], F32R, tag="ht")
        qt_sb = big.tile([P, DG // P, T], F32R, tag="qt")
        kt_sb = big.tile([P, DG // P, T], F32R, tag="kt")
        v_sb = big.tile([P, NT, DG], BF16, tag="v")
        ot_sb = big.tile([P, DG // P, T], F32R, tag="ot")

        # ================= Phase A: LayerNorm + transpose =================
        with (
            tc.tile_pool(name="ln", bufs=3) as ln,
            tc.tile_pool(name="lnst", bufs=4) as lnst,
            tc.tile_pool(name="tpp", bufs=4, space="PSUM") as tpp,
        ):
            for tb in range(NT):
                xt = ln.tile([P, D], F32, tag="xt")
                nc.sync.dma_start(out=xt, in_=x[tb * P:(tb + 1) * P, :])

                stats = lnst.tile([P, 2, 6], F32, tag="stats")
                xg = xt.rearrange("p (g d) -> p g d", g=2)
                nc.vector.bn_stats(out=stats[:, 0, :], in_=xg[:, 0, :])
                nc.vector.bn_stats(out=stats[:, 1, :], in_=xg[:, 1, :])
                mv = lnst.tile([P, 2], F32, tag="mv")
                nc.vector.bn_aggr(out=mv, in_=stats)

                rstd = lnst.tile([P, 1], F32, tag="rstd")
                nc.scalar.activation(
                    out=rstd, in_=mv[:, 1:2],
                    func=mybir.ActivationFunctionType.Sqrt,
                    bias=eps_sb, scale=1.0,
                )
                nc.vector.reciprocal(out=rstd, in_=rstd)
                nmr = lnst.tile([P, 1], F32, tag="nmr")
                nc.vector.tensor_mul(nmr, mv[:, 0:1], rstd)
                nc.vector.tensor_scalar_mul(nmr, nmr, -1.0)

                hn = ln.tile([P, D], F32R, tag="hn")
                nc.scalar.activation(
                    out=hn, in_=xt,
                    func=mybir.ActivationFunctionType.Identity,
                    bias=nmr, scale=rstd,
                )
                for c in range(KC):
                    tp = tpp.tile([P, P], F32R, tag="tp")
                    nc.tensor.transpose(
                        _rc(tp), _rc(hn[:, c * P:(c + 1) * P]), _rc(idf_sb))
                    nc.vector.tensor_copy(
                        ht_sb[:, c, tb * P:(tb + 1) * P], tp)

        # ================= Phase B: Q/K/V projections =================
        with tc.tile_pool(name="qkvp", bufs=3, space="PSUM") as qkvp:
            NQ = 512
            for oc in range(DG // P):
                for nt in range(T // NQ):
                    for w_sb, dst, b_sb in ((wq_sb, qt_sb, bq_sb),
                                            (wk_sb, kt_sb, bk_sb)):
                        ps = qkvp.tile([P, NQ], F32, tag="ps")
                        for kc in range(KC):
                            nc.tensor.matmul(
                                ps,
                                _rc(w_sb[:, kc, oc * P:(oc + 1) * P]),
                                _rc(ht_sb[:, kc, nt * NQ:(nt + 1) * NQ]),
                                start=(kc == 0), stop=(kc == KC - 1),
                            )
                        nc.vector.tensor_scalar_add(
                            dst[:, oc, nt * NQ:(nt + 1) * NQ], ps,
                            b_sb[:, oc:oc + 1])
            for tb in range(NT):
                ps = qkvp.tile([P, DG], F32, tag="psv")
                for kc in range(KC):
                    nc.tensor.matmul(
                        ps,
                        _rc(ht_sb[:, kc, tb * P:(tb + 1) * P]),
                        _rc(wv_sb[:, kc, :]),
                        start=(kc == 0), stop=(kc == KC - 1),
                    )
                nc.vector.tensor_add(v_sb[:, tb, :], ps, bv_sb)

        # ================= Phase C: banded attention =================
        with (
            tc.tile_pool(name="sp", bufs=2, space="PSUM") as sp,
            tc.tile_pool(name="ptp", bufs=3, space="PSUM") as ptp,
            tc.tile_pool(name="avp", bufs=2, space="PSUM") as avp,
            tc.tile_pool(name="smx", bufs=3) as smx,
            tc.tile_pool(name="smst", bufs=4) as smst,
        ):
            for oc in range(DG // P):           # output-chunk = head pair
                for qb in range(NT):
                    njb = min(3, qb + 1)
                    jw = njb * P
                    j0 = (qb - njb + 1) * P
                    av = avp.tile([P, P], F32, tag="av")
                    for hh in range(2):         # head within pair
                        p0 = hh * DK
                        qsl = slice(qb * P, (qb + 1) * P)
                        s = sp.tile([P, 3 * P], F32, tag="s")
                        nc.tensor.matmul(
                            s[:, :jw],
                            _rc(qt_sb[p0:p0 + DK, oc, qsl]),
                            _rc(kt_sb[p0:p0 + DK, oc, j0:j0 + jw]),
                            start=True, stop=True,
                        )
                        if njb == 3:
                            nc.vector.tensor_add(
                                s[:, 0:P], s[:, 0:P], mup_sb)
                        nc.vector.tensor_add(
                            s[:, jw - P:jw], s[:, jw - P:jw], mlo_sb)

                        pb = smx.tile([P, 3 * P], BF16, tag="pb")
                        den = smst.tile([P, 1], F32, tag="den")
                        nc.scalar.activation(
                            out=pb[:, :jw], in_=s[:, :jw],
                            func=mybir.ActivationFunctionType.Exp,
                            accum_out=den,
                        )
                        nc.vector.reciprocal(out=den, in_=den)
                        nc.vector.tensor_scalar_mul(
                            pb[:, :jw], pb[:, :jw], den)

                        h = oc * 2 + hh
                        for jj in range(njb):
                            pt = ptp.tile([P, P], BF16, tag="pt")
                            nc.tensor.transpose(
                                pt, pb[:, jj * P:(jj + 1) * P], idb_sb)
                            pts = smx.tile([P, P], BF16, tag="pts")
                            nc.vector.tensor_copy(pts, pt)
                            jb = qb - njb + 1 + jj
                            nc.tensor.matmul(
                                av[p0:p0 + DK, :],
                                v_sb[:, jb, h * DK:(h + 1) * DK],
                                pts,
                                start=(jj == 0), stop=(jj == njb - 1),
                                tile_position=(0, p0),
                            )
                    nc.vector.tensor_copy(ot_sb[:, oc, qsl], av)

        # ================= Phase D: partial out-projection =================
        with (
            tc.tile_pool(name="fpp", bufs=3, space="PSUM") as fpp,
            tc.tile_pool(name="fout", bufs=3) as fout,
        ):
            NO = 512
            for tb in range(NT):
                for on in range(D // NO):
                    ps = fpp.tile([P, NO], F32, tag="ps")
                    for kd in range(DG // P):
                        nc.tensor.matmul(
                            ps,
                            _rc(ot_sb[:, kd, tb * P:(tb + 1) * P]),
                            _rc(wo_sb[:, kd, on * NO:(on + 1) * NO]),
                            start=(kd == 0), stop=(kd == DG // P - 1),
                        )
                    ob = fout.tile([P, NO], F32, tag="ob")
                    nc.vector.tensor_copy(ob, ps)
                    nc.sync.dma_start(
                        out=partial[tb * P:(tb + 1) * P, on * NO:(on + 1) * NO],
                        in_=ob)


def build_nc():
    nc = bacc.Bacc("TRN2", target_bir_lowering=False, debug=False,
                   num_devices=8)
    with tile.TileContext(nc) as tc:
        _body(tc)
    nc.compile()
    return nc


def _prep_core_inputs(x, Wq, Wk, Wv, Wo, gamma, beta):
    """Host-side prep: per-(batch, head-group) input dicts."""
    import ml_dtypes
    B = x.shape[0]
    NEG = np.float32(MASKVAL)
    ii = np.arange(P)[:, None]
    jj = np.arange(P)[None, :]
    mup = np.where(jj > ii, np.float32(0), NEG).astype(np.float32)
    mlo = np.where(jj <= ii, np.float32(0), NEG).astype(np.float32)
    idf = np.eye(P, dtype=np.float32)
    idb = np.eye(P, dtype=np.float32).astype(ml_dtypes.bfloat16)

    in_maps = []
    for b in range(B):
        for g in range(4):
            sl = slice(g * DG, (g + 1) * DG)
            sq = np.float32(1.0 / np.sqrt(DK))
            wq_g = (gamma[:, None] * Wq[:, sl] * sq).astype(np.float32)
            wk_g = (gamma[:, None] * Wk[:, sl]).astype(np.float32)
            wv_g = (gamma[:, None] * Wv[:, sl]).astype(np.float32)
            bq_g = ((beta @ Wq[:, sl]) * sq).astype(np.float32)
            bk_g = (beta @ Wk[:, sl]).astype(np.float32)
            bv_g = (beta @ Wv[:, sl]).astype(np.float32)
            in_maps.append({
                "x": np.ascontiguousarray(x[b]).astype(np.float32),
                "wq": wq_g, "wk": wk_g, "wv": wv_g,
                "wo": np.ascontiguousarray(Wo[sl, :]).astype(np.float32),
                "bq": np.ascontiguousarray(bq_g.reshape(DG // P, P).T),
                "bk": np.ascontiguousarray(bk_g.reshape(DG // P, P).T),
                "bv": np.tile(bv_g[None, :], (P, 1)),
                "mup": mup, "mlo": mlo, "idf": idf, "idb": idb,
            })
    return in_maps


def _ntff_hook(so_path="/opt/axon/libaxon_pjrt.so"):
    import contextlib
    import ctypes

    lib = ctypes.CDLL(so_path)
    lib.axon_start_nrt_profile.argtypes = [
        ctypes.POINTER(ctypes.c_int64), ctypes.c_size_t]
    lib.axon_start_nrt_profile.restype = ctypes.c_int64
    lib.axon_stop_nrt_profile.argtypes = [ctypes.c_char_p]
    lib.axon_stop_nrt_profile.restype = ctypes.c_int64

    @contextlib.contextmanager
    def _hook(output_dir, device_ids):
        import jax
        jax.devices()
        if device_ids:
            ids = (ctypes.c_int64 * len(device_ids))(*device_ids)
            rc = lib.axon_start_nrt_profile(ids, len(device_ids))
        else:
            rc = lib.axon_start_nrt_profile(None, 0)
        if rc != 0:
            raise RuntimeError(f"axon_start_nrt_profile rc={rc}")
        try:
            yield
        finally:
            n = lib.axon_stop_nrt_profile(str(output_dir).encode())
            print(f"profile: {n} file(s) written to {output_dir}")

    return _hook


def _run_traced(nc, in_maps, trace_dir=None):
    """Execute via PJRT with NTFF capture; return BassKernelResults with
    exec_time_ns and a perfetto trace."""
    import glob
    import tempfile

    import gauge.profiler
    from concourse import bass2jax, bass_utils
    from concourse._compat import FishPath

    neff_dir = trace_dir or tempfile.mkdtemp(prefix="trn_trace_")
    hook = _ntff_hook()
    with hook(neff_dir, [0]):
        results = bass2jax.run_bass_via_pjrt(nc, in_maps, n_cores=len(in_maps))

    ntffs = glob.glob(os.path.join(neff_dir, "*_body*.ntff"))
    if not ntffs:
        print(f"no ntffs in {neff_dir}: {os.listdir(neff_dir)}")
        return bass_utils.BassKernelResults(
            results=results, instructions_and_trace=None,
            profile_json=None, exec_time_ns=None)

    profile = gauge.profiler.Profile(
        profile_path=FishPath(neff_dir),
        kernel_dev_mode=True,
        profile_on_exit=False,
        bass_kernel=nc.m,
        offline_processing=True,
        fname="*_body*",
        metadata={},
    )
    return bass_utils._process_ntff_profile(
        profile, neff_dir, nc, list(range(len(in_maps))),
        None, False, {}, trace_events=False,
    ).as_bass_kernel_results(results)


def kernel(x, Wq, Wk, Wv, Wo, bo, gamma, beta, trace=False):
    global LAST_PROFILE
    x = np.asarray(x, dtype=np.float32)
    Wq, Wk, Wv, Wo = (np.asarray(a, dtype=np.float32) for a in (Wq, Wk, Wv, Wo))
    bo = np.asarray(bo, dtype=np.float32)
    gamma = np.asarray(gamma, dtype=np.float32)
    beta = np.asarray(beta, dtype=np.float32)

    nc = build_nc()
    in_maps = _prep_core_inputs(x, Wq, Wk, Wv, Wo, gamma, beta)
    if trace:
        res = _run_traced(nc, in_maps)
    else:
        res = run_bass_kernel_spmd(nc, in_maps, core_ids=list(range(8)))
    LAST_PROFILE = {"exec_time_ns": res.exec_time_ns}

    B = x.shape[0]
    out = np.empty_like(x)
    for b in range(B):
        acc = x[b] + bo[None, :]
        for g in range(4):
            acc = acc + res.results[b * 4 + g]["partial"]
        out[b] = acc
    return out



# revision 3
# speedup vs baseline: 1.5569x; 1.5569x over previous
"""Local causal (sliding-window) attention block on 8 TRN2 NeuronCores.

Reference computation (per batch b):
    h = LayerNorm(x) * gamma + beta
    Q = h@Wq, K = h@Wk, V = h@Wv          (heads: 16 x 64)
    S = QK^T/sqrt(dk) masked to causal band of width 256
    out = x + softmax(S)@V @ Wo + bo

Sharding: 8 cores = 2 batches x 4 head-groups (4 heads each).
Each core computes LN(x_b), its head-group's Q/K/V, banded attention,
and a partial out-projection  attn_g @ Wo[g]  (token-major, [T, D]).
Host reduces: out[b] = x[b] + sum_g partial[b,g] + bo.

Attention is computed key-major: for key block kb, scores
s_t[k, q] = K_kb^T Q over the query window [kb, kb+2]; exp lands P^T
directly in SBUF (no per-block P transposes), band masking is a binary
multiply on GpSimd, and the AV matmul (lhsT = P^T slice, rhs = [V | 1])
produces token-major attention output with the softmax denominator as
column 64 -- so normalization is a per-partition scalar multiply.

gamma (and 1/sqrt(dk) for Q) are folded into the projection weights on
the host; beta enters via folded bias rows beta@W.  All matmul operands
are bf16 (PSUM accumulation in fp32).
"""

import os

import numpy as np

import concourse.bass as bass
import concourse.tile as tile
from concourse import bacc, mybir
from concourse.bass_utils import run_bass_kernel_spmd

F32 = mybir.dt.float32
BF16 = mybir.dt.bfloat16

T = 2048          # tokens per batch
D = 1024          # model dim
HG = 4            # heads per core
DK = 64           # head dim
DG = HG * DK      # head-group feature width (256)
WIN = 256         # attention window
P = 128           # partitions
NT = T // P       # 16 token tiles
KC = D // P       # 8 feature chunks
LN_EPS = 1e-5

# filled by test.py via run(trace=True)
LAST_PROFILE = {}


def _body(tc):
    nc = tc.nc

    x = nc.dram_tensor("x", [T, D], F32, kind="ExternalInput").ap()
    wq = nc.dram_tensor("wq", [D, DG], BF16, kind="ExternalInput").ap()
    wk = nc.dram_tensor("wk", [D, DG], BF16, kind="ExternalInput").ap()
    wv = nc.dram_tensor("wv", [D, DG], BF16, kind="ExternalInput").ap()
    wo = nc.dram_tensor("wo", [DG, D], BF16, kind="ExternalInput").ap()
    bq = nc.dram_tensor("bq", [P, DG // P], F32, kind="ExternalInput").ap()
    bk = nc.dram_tensor("bk", [P, DG // P], F32, kind="ExternalInput").ap()
    bv = nc.dram_tensor("bv", [P, DG], F32, kind="ExternalInput").ap()
    mka = nc.dram_tensor("mka", [P, P], BF16, kind="ExternalInput").ap()
    mkb = nc.dram_tensor("mkb", [P, P], BF16, kind="ExternalInput").ap()
    idb = nc.dram_tensor("idb", [P, P], BF16, kind="ExternalInput").ap()
    partial = nc.dram_tensor("partial", [T, D], BF16, kind="ExternalOutput").ap()

    with (
        tc.tile_pool(name="consts", bufs=1) as consts,
        tc.tile_pool(name="big", bufs=1) as big,
    ):
        # ---- resident SBUF tensors ----
        wq_sb = consts.tile([P, KC, DG], BF16, tag="wq")
        wk_sb = consts.tile([P, KC, DG], BF16, tag="wk")
        wv_sb = consts.tile([P, KC, DG], BF16, tag="wv")
        wo_sb = consts.tile([P, DG // P, D], BF16, tag="wo")
        bq_sb = consts.tile([P, DG // P], F32, tag="bq")
        bk_sb = consts.tile([P, DG // P], F32, tag="bk")
        bv_sb = consts.tile([P, DG], F32, tag="bv")
        mka_sb = consts.tile([P, P], BF16, tag="mka")
        mkb_sb = consts.tile([P, P], BF16, tag="mkb")
        idb_sb = consts.tile([P, P], BF16, tag="idb")
        eps_sb = consts.tile([P, 1], F32, tag="eps")

        nc.sync.dma_start(out=wq_sb, in_=wq.rearrange("(c p) n -> p c n", p=P))
        nc.sync.dma_start(out=wk_sb, in_=wk.rearrange("(c p) n -> p c n", p=P))
        nc.sync.dma_start(out=wv_sb, in_=wv.rearrange("(c p) n -> p c n", p=P))
        nc.sync.dma_start(out=wo_sb, in_=wo.rearrange("(c p) n -> p c n", p=P))
        nc.sync.dma_start(out=bq_sb, in_=bq)
        nc.sync.dma_start(out=bk_sb, in_=bk)
        nc.sync.dma_start(out=bv_sb, in_=bv)
        nc.sync.dma_start(out=mka_sb, in_=mka)
        nc.sync.dma_start(out=mkb_sb, in_=mkb)
        nc.sync.dma_start(out=idb_sb, in_=idb)
        nc.vector.memset(eps_sb, LN_EPS)

        # h^T (LayerNormed x, feature-major), Q^T/K^T (feature-major),
        # V (token-major, [V | 1] per head), O^T (attn out, feature-major)
        ht_sb = big.tile([P, KC, T], BF16, tag="ht")
        qt_sb = big.tile([P, DG // P, T], BF16, tag="qt")
        kt_sb = big.tile([P, DG // P, T], BF16, tag="kt")
        v_sb = big.tile([P, NT, HG, DK + 1], BF16, tag="v")
        ot_sb = big.tile([P, DG // P, T], BF16, tag="ot")

        # ones column of [V | 1]
        nc.vector.memset(v_sb[:, :, :, DK:DK + 1], 1.0)

        # ================= Phase A: LayerNorm + transpose =================
        with (
            tc.tile_pool(name="ln", bufs=3) as ln,
            tc.tile_pool(name="lnst", bufs=4) as lnst,
            tc.tile_pool(name="tpp", bufs=4, space="PSUM") as tpp,
        ):
            for tb in range(NT):
                xt = ln.tile([P, D], F32, tag="xt")
                nc.sync.dma_start(out=xt, in_=x[tb * P:(tb + 1) * P, :])

                stats = lnst.tile([P, 2, 6], F32, tag="stats")
                xg = xt.rearrange("p (g d) -> p g d", g=2)
                nc.vector.bn_stats(out=stats[:, 0, :], in_=xg[:, 0, :])
                nc.vector.bn_stats(out=stats[:, 1, :], in_=xg[:, 1, :])
                mv = lnst.tile([P, 2], F32, tag="mv")
                nc.vector.bn_aggr(out=mv, in_=stats)

                rstd = lnst.tile([P, 1], F32, tag="rstd")
                nc.scalar.activation(
                    out=rstd, in_=mv[:, 1:2],
                    func=mybir.ActivationFunctionType.Sqrt,
                    bias=eps_sb, scale=1.0,
                )
                nc.vector.reciprocal(out=rstd, in_=rstd)
                nmr = lnst.tile([P, 1], F32, tag="nmr")
                nc.vector.tensor_mul(nmr, mv[:, 0:1], rstd)
                nc.vector.tensor_scalar_mul(nmr, nmr, -1.0)

                hn = ln.tile([P, D], BF16, tag="hn")
                nc.scalar.activation(
                    out=hn, in_=xt,
                    func=mybir.ActivationFunctionType.Identity,
                    bias=nmr, scale=rstd,
                )
                for half in range(2):
                    tp = tpp.tile([P, 4 * P], BF16, tag="tp")
                    for c4 in range(4):
                        c = half * 4 + c4
                        nc.tensor.transpose(
                            tp[:, c4 * P:(c4 + 1) * P],
                            hn[:, c * P:(c + 1) * P], idb_sb)
                    dst = ht_sb[:, half * 4:(half + 1) * 4,
                                tb * P:(tb + 1) * P]
                    if half == 0:
                        nc.vector.tensor_copy(dst, tp)
                    else:
                        nc.scalar.copy(dst, tp)

        # ================= Phase B: Q/K/V projections =================
        with tc.tile_pool(name="qkvp", bufs=3, space="PSUM") as qkvp:
            NQ = 512
            for oc in range(DG // P):
                for nt in range(T // NQ):
                    tsl = slice(nt * NQ, (nt + 1) * NQ)
                    for w_sb, dst, b_sb, on_act in (
                            (wq_sb, qt_sb, bq_sb, True),
                            (wk_sb, kt_sb, bk_sb, False)):
                        ps = qkvp.tile([P, NQ], F32, tag="ps")
                        for kc in range(KC):
                            nc.tensor.matmul(
                                ps,
                                w_sb[:, kc, oc * P:(oc + 1) * P],
                                ht_sb[:, kc, tsl],
                                start=(kc == 0), stop=(kc == KC - 1),
                            )
                        if on_act:
                            nc.scalar.activation(
                                out=dst[:, oc, tsl], in_=ps,
                                func=mybir.ActivationFunctionType.Identity,
                                bias=b_sb[:, oc:oc + 1], scale=1.0,
                            )
                        else:
                            nc.vector.tensor_scalar_add(
                                dst[:, oc, tsl], ps, b_sb[:, oc:oc + 1])
            for tb in range(NT):
                ps = qkvp.tile([P, DG], F32, tag="psv")
                for kc in range(KC):
                    nc.tensor.matmul(
                        ps,
                        ht_sb[:, kc, tb * P:(tb + 1) * P],
                        wv_sb[:, kc, :],
                        start=(kc == 0), stop=(kc == KC - 1),
                    )
                nc.vector.tensor_add(
                    v_sb[:, tb, :, 0:DK],
                    ps.rearrange("p (h d) -> p h d", h=HG), bv_sb)

        # ================= Phase C: banded attention (key-major) ==========
        with (
            tc.tile_pool(name="sp", bufs=2, space="PSUM") as sp,
            tc.tile_pool(name="avp", bufs=4, space="PSUM") as avp,
            tc.tile_pool(name="otp", bufs=2, space="PSUM") as otp,
            tc.tile_pool(name="smx", bufs=3) as smx,
            tc.tile_pool(name="smst", bufs=6) as smst,
        ):
            for oc in range(DG // P):           # head pair
                for hh in range(2):             # head within pair
                    p0 = hh * DK
                    h = oc * 2 + hh
                    avs = {}
                    otps = None
                    for kb in range(NT):
                        njb = min(3, NT - kb)
                        jw = njb * P
                        # scores s_t[k, q-window], then P^T = exp(s_t)
                        st = sp.tile([P, 3 * P], F32, tag="st")
                        nc.tensor.matmul(
                            st[:, :jw],
                            kt_sb[p0:p0 + DK, oc, kb * P:(kb + 1) * P],
                            qt_sb[p0:p0 + DK, oc, kb * P:kb * P + jw],
                            start=True, stop=True,
                        )
                        pt = smx.tile([P, 3 * P], BF16, tag="pt")
                        nc.scalar.activation(
                            out=pt[:, :jw], in_=st[:, :jw],
                            func=mybir.ActivationFunctionType.Exp,
                        )
                        # band mask: diag block keeps k<=q, oldest block
                        # keeps k>q (window 256 = 2 full blocks back)
                        nc.gpsimd.tensor_mul(
                            pt[:, 0:P], pt[:, 0:P], mka_sb)
                        if njb == 3:
                            nc.gpsimd.tensor_mul(
                                pt[:, 2 * P:3 * P], pt[:, 2 * P:3 * P],
                                mkb_sb)
                        # AV: token-major, denominator in column DK
                        for qoff in range(njb):
                            qb = kb + qoff
                            if qb not in avs:
                                avs[qb] = avp.tile([P, DK + 1], F32,
                                                   tag="av",
                                                   name=f"av_{h}_{qb}")
                            nc.tensor.matmul(
                                avs[qb],
                                pt[:, qoff * P:(qoff + 1) * P],
                                v_sb[:, kb, h, :],
                                start=(kb == max(qb - 2, 0)),
                                stop=(kb == qb),
                            )
                        # query block kb is now complete: normalize
                        av = avs.pop(kb)
                        rden = smst.tile([P, 1], F32, tag="rden")
                        nc.vector.reciprocal(out=rden, in_=av[:, DK:DK + 1])
                        o_tok = smx.tile([P, DK], BF16, tag="otok")
                        nc.vector.tensor_scalar_mul(o_tok, av[:, 0:DK], rden)
                        # transpose back to feature-major, 4 blocks a batch
                        if kb % 4 == 0:
                            otps = otp.tile([DK, 4 * P], BF16, tag="otb")
                        nc.tensor.transpose(
                            otps[:, (kb % 4) * P:(kb % 4 + 1) * P],
                            o_tok, idb_sb)
                        if kb % 4 == 3:
                            g = kb // 4
                            dst = ot_sb[p0:p0 + DK, oc,
                                        g * 4 * P:(g + 1) * 4 * P]
                            if (h % 2) == 0:
                                nc.vector.tensor_copy(dst, otps)
                            else:
                                nc.scalar.copy(dst, otps)

        # ================= Phase D: partial out-projection =================
        with (
            tc.tile_pool(name="fpp", bufs=3, space="PSUM") as fpp,
            tc.tile_pool(name="fout", bufs=4) as fout,
        ):
            NO = 512
            for tb in range(NT):
                for on in range(D // NO):
                    ps = fpp.tile([P, NO], F32, tag="ps")
                    for kd in range(DG // P):
                        nc.tensor.matmul(
                            ps,
                            ot_sb[:, kd, tb * P:(tb + 1) * P],
                            wo_sb[:, kd, on * NO:(on + 1) * NO],
                            start=(kd == 0), stop=(kd == DG // P - 1),
                        )
                    ob = fout.tile([P, NO], BF16, tag="ob")
                    if on == 0:
                        nc.vector.tensor_copy(ob, ps)
                    else:
                        nc.scalar.copy(ob, ps)
                    nc.sync.dma_start(
                        out=partial[tb * P:(tb + 1) * P, on * NO:(on + 1) * NO],
                        in_=ob)


def build_nc():
    nc = bacc.Bacc("TRN2", target_bir_lowering=False, debug=False,
                   num_devices=8)
    with tile.TileContext(nc) as tc:
        _body(tc)
    nc.compile()
    return nc


def _prep_core_inputs(x, Wq, Wk, Wv, Wo, gamma, beta):
    """Host-side prep: per-(batch, head-group) input dicts."""
    import ml_dtypes
    BF = ml_dtypes.bfloat16
    B = x.shape[0]
    ii = np.arange(P)[:, None]   # key index within block (rows)
    jj = np.arange(P)[None, :]   # query index within block (cols)
    # diag block: keep k <= q ; oldest block: keep k > q
    mka_np = (ii <= jj).astype(BF)
    mkb_np = (ii > jj).astype(BF)
    idb_np = np.eye(P, dtype=np.float32).astype(BF)

    in_maps = []
    for b in range(B):
        for g in range(4):
            sl = slice(g * DG, (g + 1) * DG)
            sq = np.float32(1.0 / np.sqrt(DK))
            wq_g = (gamma[:, None] * Wq[:, sl] * sq).astype(BF)
            wk_g = (gamma[:, None] * Wk[:, sl]).astype(BF)
            wv_g = (gamma[:, None] * Wv[:, sl]).astype(BF)
            bq_g = ((beta @ Wq[:, sl]) * sq).astype(np.float32)
            bk_g = (beta @ Wk[:, sl]).astype(np.float32)
            bv_g = (beta @ Wv[:, sl]).astype(np.float32)
            in_maps.append({
                "x": np.ascontiguousarray(x[b]).astype(np.float32),
                "wq": wq_g, "wk": wk_g, "wv": wv_g,
                "wo": np.ascontiguousarray(Wo[sl, :]).astype(BF),
                "bq": np.ascontiguousarray(bq_g.reshape(DG // P, P).T),
                "bk": np.ascontiguousarray(bk_g.reshape(DG // P, P).T),
                "bv": np.tile(bv_g[None, :], (P, 1)),
                "mka": mka_np, "mkb": mkb_np, "idb": idb_np,
            })
    return in_maps


def _ntff_hook(so_path="/opt/axon/libaxon_pjrt.so"):
    import contextlib
    import ctypes

    lib = ctypes.CDLL(so_path)
    lib.axon_start_nrt_profile.argtypes = [
        ctypes.POINTER(ctypes.c_int64), ctypes.c_size_t]
    lib.axon_start_nrt_profile.restype = ctypes.c_int64
    lib.axon_stop_nrt_profile.argtypes = [ctypes.c_char_p]
    lib.axon_stop_nrt_profile.restype = ctypes.c_int64

    @contextlib.contextmanager
    def _hook(output_dir, device_ids):
        import jax
        jax.devices()
        if device_ids:
            ids = (ctypes.c_int64 * len(device_ids))(*device_ids)
            rc = lib.axon_start_nrt_profile(ids, len(device_ids))
        else:
            rc = lib.axon_start_nrt_profile(None, 0)
        if rc != 0:
            raise RuntimeError(f"axon_start_nrt_profile rc={rc}")
        try:
            yield
        finally:
            n = lib.axon_stop_nrt_profile(str(output_dir).encode())
            print(f"profile: {n} file(s) written to {output_dir}")

    return _hook


def _run_traced(nc, in_maps, trace_dir=None):
    """Execute via PJRT with NTFF capture; return BassKernelResults with
    exec_time_ns and a perfetto trace."""
    import glob
    import tempfile

    import gauge.profiler
    from concourse import bass2jax, bass_utils
    from concourse._compat import FishPath

    neff_dir = trace_dir or tempfile.mkdtemp(prefix="trn_trace_")
    hook = _ntff_hook()
    with hook(neff_dir, [0]):
        results = bass2jax.run_bass_via_pjrt(nc, in_maps, n_cores=len(in_maps))

    ntffs = glob.glob(os.path.join(neff_dir, "*_body*.ntff"))
    if not ntffs:
        print(f"no ntffs in {neff_dir}: {os.listdir(neff_dir)}")
        return bass_utils.BassKernelResults(
            results=results, instructions_and_trace=None,
            profile_json=None, exec_time_ns=None)

    profile = gauge.profiler.Profile(
        profile_path=FishPath(neff_dir),
        kernel_dev_mode=True,
        profile_on_exit=False,
        bass_kernel=nc.m,
        offline_processing=True,
        fname="*_body*",
        metadata={},
    )
    return bass_utils._process_ntff_profile(
        profile, neff_dir, nc, list(range(len(in_maps))),
        None, False, {}, trace_events=False,
    ).as_bass_kernel_results(results)


def kernel(x, Wq, Wk, Wv, Wo, bo, gamma, beta, trace=False):
    global LAST_PROFILE
    x = np.asarray(x, dtype=np.float32)
    Wq, Wk, Wv, Wo = (np.asarray(a, dtype=np.float32) for a in (Wq, Wk, Wv, Wo))
    bo = np.asarray(bo, dtype=np.float32)
    gamma = np.asarray(gamma, dtype=np.float32)
    beta = np.asarray(beta, dtype=np.float32)

    nc = build_nc()
    in_maps = _prep_core_inputs(x, Wq, Wk, Wv, Wo, gamma, beta)
    if trace:
        res = _run_traced(nc, in_maps)
    else:
        res = run_bass_kernel_spmd(nc, in_maps, core_ids=list(range(8)))
    LAST_PROFILE = {"exec_time_ns": res.exec_time_ns}

    B = x.shape[0]
    out = np.empty_like(x)
    for b in range(B):
        acc = x[b] + bo[None, :]
        for g in range(4):
            acc = acc + res.results[b * 4 + g]["partial"].astype(np.float32)
        out[b] = acc
    return out


# revision 11
# speedup vs baseline: 1.8800x; 1.2075x over previous
"""Local causal (sliding-window) attention block on 8 TRN2 NeuronCores.

Reference computation (per batch b):
    h = LayerNorm(x) * gamma + beta
    Q = h@Wq, K = h@Wk, V = h@Wv          (heads: 16 x 64)
    S = QK^T/sqrt(dk) masked to causal band of width 256
    out = x + softmax(S)@V @ Wo + bo

Sharding: 8 cores = 2 batches x 4 head-groups (4 heads each).
Each core computes LN(x_b), its head-group's Q/K/V, banded attention,
and a partial out-projection  attn_g @ Wo[g]  (token-major, [T, D]).
Host reduces: out[b] = x[b] + sum_g partial[b,g] + bo.

Attention is computed key-major: for key block kb, scores
s_t[k, q] = K_kb^T Q over the query window [kb, kb+2]; exp lands P^T
directly in SBUF (no per-block P transposes), band masking is a binary
multiply on GpSimd, and the AV matmul (lhsT = P^T slice, rhs = [V | 1])
produces token-major attention output with the softmax denominator as
column 64 -- so normalization is a per-partition scalar multiply.

gamma (and 1/sqrt(dk) for Q) are folded into the projection weights on
the host; beta enters via folded bias rows beta@W.  All matmul operands
are bf16 (PSUM accumulation in fp32).
"""

import os

import numpy as np

import concourse.bass as bass
import concourse.tile as tile
from concourse import bacc, mybir
from concourse.bass_utils import run_bass_kernel_spmd

F32 = mybir.dt.float32
BF16 = mybir.dt.bfloat16

T = 2048          # tokens per batch
D = 1024          # model dim
HG = 4            # heads per core
DK = 64           # head dim
DG = HG * DK      # head-group feature width (256)
WIN = 256         # attention window
P = 128           # partitions
NT = T // P       # 16 token tiles
KC = D // P       # 8 feature chunks
LN_EPS = 1e-5

# filled by test.py via run(trace=True)
LAST_PROFILE = {}


def _body(tc):
    nc = tc.nc

    x = nc.dram_tensor("x", [T, D], F32, kind="ExternalInput").ap()
    wq = nc.dram_tensor("wq", [D, DG], BF16, kind="ExternalInput").ap()
    wk = nc.dram_tensor("wk", [D, DG], BF16, kind="ExternalInput").ap()
    wv = nc.dram_tensor("wv", [D, DG], BF16, kind="ExternalInput").ap()
    wo = nc.dram_tensor("wo", [DG, D], BF16, kind="ExternalInput").ap()
    bq = nc.dram_tensor("bq", [P, DG // P], F32, kind="ExternalInput").ap()
    bk = nc.dram_tensor("bk", [P, DG // P], F32, kind="ExternalInput").ap()
    bv = nc.dram_tensor("bv", [P, DG], F32, kind="ExternalInput").ap()
    msk = nc.dram_tensor("msk", [P, 3 * P], BF16, kind="ExternalInput").ap()
    idb = nc.dram_tensor("idb", [P, P], BF16, kind="ExternalInput").ap()
    partial = nc.dram_tensor("partial", [T, D], BF16, kind="ExternalOutput").ap()

    with (
        tc.tile_pool(name="consts", bufs=1) as consts,
        tc.tile_pool(name="big", bufs=1) as big,
    ):
        # ---- resident SBUF tensors ----
        wq_sb = consts.tile([P, KC, DG], BF16, tag="wq")
        wk_sb = consts.tile([P, KC, DG], BF16, tag="wk")
        wv_sb = consts.tile([P, KC, DG], BF16, tag="wv")
        wo_sb = consts.tile([P, DG // P, D], BF16, tag="wo")
        bq_sb = consts.tile([P, DG // P], F32, tag="bq")
        bk_sb = consts.tile([P, DG // P], F32, tag="bk")
        bv_sb = consts.tile([P, DG], F32, tag="bv")
        msk_sb = consts.tile([P, 3 * P], BF16, tag="msk")
        idb_sb = consts.tile([P, P], BF16, tag="idb")
        eps_sb = consts.tile([P, 1], F32, tag="eps")

        nc.sync.dma_start(out=idb_sb, in_=idb)
        nc.sync.dma_start(out=msk_sb, in_=msk)
        nc.vector.memset(eps_sb, LN_EPS)

        # h^T (LayerNormed x, feature-major), Q^T/K^T (feature-major),
        # V (token-major, [V | 1] per head), O^T (attn out, feature-major)
        ht_sb = big.tile([P, KC, T], BF16, tag="ht")
        qt_sb = big.tile([P, DG // P, T], BF16, tag="qt")
        kt_sb = big.tile([P, DG // P, T], BF16, tag="kt")
        v_sb = big.tile([P, NT, HG, DK + 1], BF16, tag="v")
        ot_sb = big.tile([P, DG // P, T], BF16, tag="ot")

        # ones column of [V | 1]
        nc.vector.memset(v_sb[:, :, :, DK:DK + 1], 1.0)

        # ===== Phase A+B interleaved: LayerNorm + transpose + Q/K/V =====
        # A is DVE/ACT-bound, B is PE-bound; interleaving the emission lets
        # the projection matmuls of token chunk nt backfill the PE while
        # LayerNorm of later tiles runs on the other engines.
        with (
            tc.tile_pool(name="ln", bufs=3) as ln,
            tc.tile_pool(name="lnst", bufs=4) as lnst,
            tc.tile_pool(name="abp", bufs=2, space="PSUM") as abp,
            tc.tile_pool(name="tpp", bufs=3, space="PSUM") as tpp,
        ):
            NQ = 512
            for tb in range(NT):
                xt = ln.tile([P, D], F32, tag="xt")
                nc.sync.dma_start(out=xt, in_=x[tb * P:(tb + 1) * P, :])
                if tb == 0:
                    # weights are first needed by the V projection below;
                    # issuing them after x(0) keeps the LN pipe from
                    # waiting on the strided weight loads.
                    nc.sync.dma_start(
                        out=wq_sb, in_=wq.rearrange("(c p) n -> p c n", p=P))
                    nc.sync.dma_start(
                        out=wk_sb, in_=wk.rearrange("(c p) n -> p c n", p=P))
                    nc.sync.dma_start(
                        out=wv_sb, in_=wv.rearrange("(c p) n -> p c n", p=P))
                    nc.sync.dma_start(
                        out=wo_sb, in_=wo.rearrange("(c p) n -> p c n", p=P))
                    nc.sync.dma_start(out=bq_sb, in_=bq)
                    nc.sync.dma_start(out=bk_sb, in_=bk)
                    nc.sync.dma_start(out=bv_sb, in_=bv)

                stats = lnst.tile([P, 2, 6], F32, tag="stats")
                xg = xt.rearrange("p (g d) -> p g d", g=2)
                nc.vector.bn_stats(out=stats[:, 0, :], in_=xg[:, 0, :])
                nc.vector.bn_stats(out=stats[:, 1, :], in_=xg[:, 1, :])
                mv = lnst.tile([P, 2], F32, tag="mv")
                nc.vector.bn_aggr(out=mv, in_=stats)

                rstd = lnst.tile([P, 1], F32, tag="rstd")
                nc.scalar.activation(
                    out=rstd, in_=mv[:, 1:2],
                    func=mybir.ActivationFunctionType.Sqrt,
                    bias=eps_sb, scale=1.0,
                )
                nc.vector.reciprocal(out=rstd, in_=rstd)
                nmr = lnst.tile([P, 1], F32, tag="nmr")
                nc.vector.scalar_tensor_tensor(
                    nmr, mv[:, 0:1], -1.0, rstd,
                    mybir.AluOpType.mult, mybir.AluOpType.mult)

                hn = ln.tile([P, D], BF16, tag="hn")
                nc.gpsimd.tensor_scalar(
                    hn, xt, rstd, nmr,
                    mybir.AluOpType.mult, mybir.AluOpType.add)
                for half in range(2):
                    tp = tpp.tile([P, 4 * P], BF16, tag="tp")
                    for c4 in range(4):
                        c = half * 4 + c4
                        nc.tensor.transpose(
                            tp[:, c4 * P:(c4 + 1) * P],
                            hn[:, c * P:(c + 1) * P], idb_sb)
                    dst = ht_sb[:, half * 4:(half + 1) * 4,
                                tb * P:(tb + 1) * P]
                    nc.scalar.copy(dst, tp)

                # V projection for this tile
                psv = abp.tile([P, DG], F32, tag="psv")
                for kc in range(KC):
                    nc.tensor.matmul(
                        psv,
                        ht_sb[:, kc, tb * P:(tb + 1) * P],
                        wv_sb[:, kc, :],
                        start=(kc == 0), stop=(kc == KC - 1),
                    )
                nc.vector.tensor_add(
                    v_sb[:, tb, :, 0:DK],
                    psv.rearrange("p (h d) -> p h d", h=HG), bv_sb)

                # Q/K projections for the completed 512-token chunk
                if tb % 4 == 3:
                    nt = tb // 4
                    tsl = slice(nt * NQ, (nt + 1) * NQ)
                    for oc in range(DG // P):
                        for w_sb, dst, b_sb, on_act in (
                                (wq_sb, qt_sb, bq_sb, True),
                                (wk_sb, kt_sb, bk_sb, False)):
                            ps = abp.tile([P, NQ], F32, tag="ps")
                            for kc in range(KC):
                                nc.tensor.matmul(
                                    ps,
                                    w_sb[:, kc, oc * P:(oc + 1) * P],
                                    ht_sb[:, kc, tsl],
                                    start=(kc == 0), stop=(kc == KC - 1),
                                )
                            if on_act:
                                nc.scalar.activation(
                                    out=dst[:, oc, tsl], in_=ps,
                                    func=mybir.ActivationFunctionType.Identity,
                                    bias=b_sb[:, oc:oc + 1], scale=1.0,
                                )
                            else:
                                nc.vector.tensor_scalar_add(
                                    dst[:, oc, tsl], ps, b_sb[:, oc:oc + 1])

        # ================= Phase C: banded attention (key-major) ==========
        with (
            tc.tile_pool(name="sp", bufs=2, space="PSUM") as sp,
            tc.tile_pool(name="avp", bufs=4, space="PSUM") as avp,
            tc.tile_pool(name="otp", bufs=2, space="PSUM") as otp,
            tc.tile_pool(name="smx", bufs=3) as smx,
            tc.tile_pool(name="smst", bufs=6) as smst,
        ):
            for oc in range(DG // P):           # head pair
                for hh in range(2):             # head within pair
                    p0 = hh * DK
                    h = oc * 2 + hh
                    avs = {}
                    pts = {}
                    otps_box = [None]

                    def scores(kb):
                        # s_t[k, q-window]; exp lands P^T in SBUF; band
                        # mask (diag keeps k<=q, oldest block keeps k>q)
                        # is one binary multiply against [mka | 1 | mkb]
                        njb = min(3, NT - kb)
                        jw = njb * P
                        st = sp.tile([P, 3 * P], F32, tag="st",
                                     name=f"st_{h}_{kb}")
                        nc.tensor.matmul(
                            st[:, :jw],
                            kt_sb[p0:p0 + DK, oc, kb * P:(kb + 1) * P],
                            qt_sb[p0:p0 + DK, oc, kb * P:kb * P + jw],
                            start=True, stop=True,
                        )
                        pt = smx.tile([P, 3 * P], BF16, tag="pt",
                                      name=f"pt_{h}_{kb}")
                        nc.scalar.activation(
                            out=pt[:, :jw], in_=st[:, :jw],
                            func=mybir.ActivationFunctionType.Exp,
                        )
                        nc.gpsimd.tensor_mul(
                            pt[:, :jw], pt[:, :jw], msk_sb[:, :jw])
                        pts[kb] = pt

                    def consume(kb):
                        # AV: token-major, denominator in column DK
                        njb = min(3, NT - kb)
                        pt = pts.pop(kb)
                        for qoff in range(njb):
                            qb = kb + qoff
                            if qb not in avs:
                                avs[qb] = avp.tile([P, DK + 1], F32,
                                                   tag="av",
                                                   name=f"av_{h}_{qb}")
                            nc.tensor.matmul(
                                avs[qb],
                                pt[:, qoff * P:(qoff + 1) * P],
                                v_sb[:, kb, h, :],
                                start=(kb == max(qb - 2, 0)),
                                stop=(kb == qb),
                            )
                        # query block kb is now complete: normalize
                        av = avs.pop(kb)
                        rden = smst.tile([P, 1], F32, tag="rden")
                        nc.vector.reciprocal(out=rden, in_=av[:, DK:DK + 1])
                        o_tok = smx.tile([P, DK], BF16, tag="otok")
                        nc.vector.tensor_scalar_mul(o_tok, av[:, 0:DK], rden)
                        # transpose back to feature-major, 4 blocks a batch
                        nonlocal_otps = otps_box
                        if kb % 4 == 0:
                            nonlocal_otps[0] = otp.tile(
                                [DK, 4 * P], BF16, tag="otb",
                                name=f"otb_{h}_{kb}")
                        nc.tensor.transpose(
                            nonlocal_otps[0][:, (kb % 4) * P:(kb % 4 + 1) * P],
                            o_tok, idb_sb)
                        if kb % 4 == 3:
                            g = kb // 4
                            dst = ot_sb[p0:p0 + DK, oc,
                                        g * 4 * P:(g + 1) * 4 * P]
                            if (h % 2) == 0:
                                nc.vector.tensor_copy(dst, nonlocal_otps[0])
                            else:
                                nc.scalar.copy(dst, nonlocal_otps[0])

                    # 1-deep software pipeline: scores(kb+1) is emitted
                    # before the AV of kb so the PE always has an
                    # independent matmul ready while exp/mask complete.
                    scores(0)
                    for kb in range(NT):
                        if kb + 1 < NT:
                            scores(kb + 1)
                        consume(kb)

        # ================= Phase D: partial out-projection =================
        with (
            tc.tile_pool(name="fpp", bufs=3, space="PSUM") as fpp,
            tc.tile_pool(name="fout", bufs=4) as fout,
        ):
            NO = 512
            for tb in range(NT):
                for on in range(D // NO):
                    ps = fpp.tile([P, NO], F32, tag="ps")
                    for kd in range(DG // P):
                        nc.tensor.matmul(
                            ps,
                            ot_sb[:, kd, tb * P:(tb + 1) * P],
                            wo_sb[:, kd, on * NO:(on + 1) * NO],
                            start=(kd == 0), stop=(kd == DG // P - 1),
                        )
                    ob = fout.tile([P, NO], BF16, tag="ob")
                    if on == 0:
                        nc.vector.tensor_copy(ob, ps)
                    else:
                        nc.scalar.copy(ob, ps)
                    nc.sync.dma_start(
                        out=partial[tb * P:(tb + 1) * P, on * NO:(on + 1) * NO],
                        in_=ob)


def build_nc():
    nc = bacc.Bacc("TRN2", target_bir_lowering=False, debug=False,
                   num_devices=8)
    with tile.TileContext(nc) as tc:
        _body(tc)
    nc.compile()
    return nc


def _prep_core_inputs(x, Wq, Wk, Wv, Wo, gamma, beta):
    """Host-side prep: per-(batch, head-group) input dicts."""
    import ml_dtypes
    BF = ml_dtypes.bfloat16
    B = x.shape[0]
    ii = np.arange(P)[:, None]   # key index within block (rows)
    jj = np.arange(P)[None, :]   # query index within block (cols)
    # diag block: keep k <= q ; middle: all ; oldest block: keep k > q
    msk_np = np.concatenate(
        [(ii <= jj), np.ones((P, P), dtype=bool), (ii > jj)],
        axis=1).astype(BF)
    idb_np = np.eye(P, dtype=np.float32).astype(BF)

    in_maps = []
    for b in range(B):
        for g in range(4):
            sl = slice(g * DG, (g + 1) * DG)
            sq = np.float32(1.0 / np.sqrt(DK))
            wq_g = (gamma[:, None] * Wq[:, sl] * sq).astype(BF)
            wk_g = (gamma[:, None] * Wk[:, sl]).astype(BF)
            wv_g = (gamma[:, None] * Wv[:, sl]).astype(BF)
            bq_g = ((beta @ Wq[:, sl]) * sq).astype(np.float32)
            bk_g = (beta @ Wk[:, sl]).astype(np.float32)
            bv_g = (beta @ Wv[:, sl]).astype(np.float32)
            in_maps.append({
                "x": np.ascontiguousarray(x[b]).astype(np.float32),
                "wq": wq_g, "wk": wk_g, "wv": wv_g,
                "wo": np.ascontiguousarray(Wo[sl, :]).astype(BF),
                "bq": np.ascontiguousarray(bq_g.reshape(DG // P, P).T),
                "bk": np.ascontiguousarray(bk_g.reshape(DG // P, P).T),
                "bv": np.tile(bv_g[None, :], (P, 1)),
                "msk": msk_np, "idb": idb_np,
            })
    return in_maps


def _ntff_hook(so_path="/opt/axon/libaxon_pjrt.so"):
    import contextlib
    import ctypes

    lib = ctypes.CDLL(so_path)
    lib.axon_start_nrt_profile.argtypes = [
        ctypes.POINTER(ctypes.c_int64), ctypes.c_size_t]
    lib.axon_start_nrt_profile.restype = ctypes.c_int64
    lib.axon_stop_nrt_profile.argtypes = [ctypes.c_char_p]
    lib.axon_stop_nrt_profile.restype = ctypes.c_int64

    @contextlib.contextmanager
    def _hook(output_dir, device_ids):
        import jax
        jax.devices()
        if device_ids:
            ids = (ctypes.c_int64 * len(device_ids))(*device_ids)
            rc = lib.axon_start_nrt_profile(ids, len(device_ids))
        else:
            rc = lib.axon_start_nrt_profile(None, 0)
        if rc != 0:
            raise RuntimeError(f"axon_start_nrt_profile rc={rc}")
        try:
            yield
        finally:
            n = lib.axon_stop_nrt_profile(str(output_dir).encode())
            print(f"profile: {n} file(s) written to {output_dir}")

    return _hook


def _run_traced(nc, in_maps, trace_dir=None):
    """Execute via PJRT with NTFF capture; return BassKernelResults with
    exec_time_ns and a perfetto trace."""
    import glob
    import tempfile

    import gauge.profiler
    from concourse import bass2jax, bass_utils
    from concourse._compat import FishPath

    neff_dir = trace_dir or tempfile.mkdtemp(prefix="trn_trace_")
    hook = _ntff_hook()
    with hook(neff_dir, [0]):
        results = bass2jax.run_bass_via_pjrt(nc, in_maps, n_cores=len(in_maps))

    ntffs = glob.glob(os.path.join(neff_dir, "*_body*.ntff"))
    if not ntffs:
        print(f"no ntffs in {neff_dir}: {os.listdir(neff_dir)}")
        return bass_utils.BassKernelResults(
            results=results, instructions_and_trace=None,
            profile_json=None, exec_time_ns=None)

    profile = gauge.profiler.Profile(
        profile_path=FishPath(neff_dir),
        kernel_dev_mode=True,
        profile_on_exit=False,
        bass_kernel=nc.m,
        offline_processing=True,
        fname="*_body*",
        metadata={},
    )
    return bass_utils._process_ntff_profile(
        profile, neff_dir, nc, list(range(len(in_maps))),
        None, False, {}, trace_events=False,
    ).as_bass_kernel_results(results)


def kernel(x, Wq, Wk, Wv, Wo, bo, gamma, beta, trace=False):
    global LAST_PROFILE
    x = np.asarray(x, dtype=np.float32)
    Wq, Wk, Wv, Wo = (np.asarray(a, dtype=np.float32) for a in (Wq, Wk, Wv, Wo))
    bo = np.asarray(bo, dtype=np.float32)
    gamma = np.asarray(gamma, dtype=np.float32)
    beta = np.asarray(beta, dtype=np.float32)

    nc = build_nc()
    in_maps = _prep_core_inputs(x, Wq, Wk, Wv, Wo, gamma, beta)
    if trace:
        res = _run_traced(nc, in_maps)
    else:
        res = run_bass_kernel_spmd(nc, in_maps, core_ids=list(range(8)))
    LAST_PROFILE = {"exec_time_ns": res.exec_time_ns}

    B = x.shape[0]
    out = np.empty_like(x)
    for b in range(B):
        acc = x[b] + bo[None, :]
        for g in range(4):
            acc = acc + res.results[b * 4 + g]["partial"].astype(np.float32)
        out[b] = acc
    return out


# revision 13
# speedup vs baseline: 1.9263x; 1.0247x over previous
"""Local causal (sliding-window) attention block on 8 TRN2 NeuronCores.

Reference computation (per batch b):
    h = LayerNorm(x) * gamma + beta
    Q = h@Wq, K = h@Wk, V = h@Wv          (heads: 16 x 64)
    S = QK^T/sqrt(dk) masked to causal band of width 256
    out = x + softmax(S)@V @ Wo + bo

Sharding: 8 cores = 2 batches x 4 head-groups (4 heads each).
Each core computes LN(x_b), its head-group's Q/K/V, banded attention,
and a partial out-projection  attn_g @ Wo[g]  (token-major, [T, D]).
Host reduces: out[b] = x[b] + sum_g partial[b,g] + bo.

Attention is computed key-major: for key block kb, scores
s_t[k, q] = K_kb^T Q over the query window [kb, kb+2]; exp lands P^T
directly in SBUF (no per-block P transposes), band masking is a binary
multiply on GpSimd, and the AV matmul (lhsT = P^T slice, rhs = [V | 1])
produces token-major attention output with the softmax denominator as
column 64 -- so normalization is a per-partition scalar multiply.

gamma (and 1/sqrt(dk) for Q) are folded into the projection weights on
the host; beta enters via folded bias rows beta@W.  All matmul operands
are bf16 (PSUM accumulation in fp32).
"""

import os

import numpy as np

import concourse.bass as bass
import concourse.tile as tile
from concourse import bacc, mybir
from concourse.bass_utils import run_bass_kernel_spmd

F32 = mybir.dt.float32
BF16 = mybir.dt.bfloat16

T = 2048          # tokens per batch
D = 1024          # model dim
HG = 4            # heads per core
DK = 64           # head dim
DG = HG * DK      # head-group feature width (256)
WIN = 256         # attention window
P = 128           # partitions
NT = T // P       # 16 token tiles
KC = D // P       # 8 feature chunks
LN_EPS = 1e-5

# filled by test.py via run(trace=True)
LAST_PROFILE = {}


def _body(tc):
    nc = tc.nc

    x = nc.dram_tensor("x", [T, D], F32, kind="ExternalInput").ap()
    wq = nc.dram_tensor("wq", [D, DG], BF16, kind="ExternalInput").ap()
    wk = nc.dram_tensor("wk", [D, DG], BF16, kind="ExternalInput").ap()
    wv = nc.dram_tensor("wv", [D, DG], BF16, kind="ExternalInput").ap()
    wo = nc.dram_tensor("wo", [DG, D], BF16, kind="ExternalInput").ap()
    bq = nc.dram_tensor("bq", [P, DG // P], F32, kind="ExternalInput").ap()
    bk = nc.dram_tensor("bk", [P, DG // P], F32, kind="ExternalInput").ap()
    bv = nc.dram_tensor("bv", [P, DG], F32, kind="ExternalInput").ap()
    msk = nc.dram_tensor("msk", [P, 3 * P], BF16, kind="ExternalInput").ap()
    idb = nc.dram_tensor("idb", [P, P], BF16, kind="ExternalInput").ap()
    partial = nc.dram_tensor("partial", [T, D], BF16, kind="ExternalOutput").ap()

    with (
        tc.tile_pool(name="consts", bufs=1) as consts,
        tc.tile_pool(name="big", bufs=1) as big,
    ):
        # ---- resident SBUF tensors ----
        wq_sb = consts.tile([P, KC, DG], BF16, tag="wq")
        wk_sb = consts.tile([P, KC, DG], BF16, tag="wk")
        wv_sb = consts.tile([P, KC, DG], BF16, tag="wv")
        wo_sb = consts.tile([P, DG // P, D], BF16, tag="wo")
        bq_sb = consts.tile([P, DG // P], F32, tag="bq")
        bk_sb = consts.tile([P, DG // P], F32, tag="bk")
        bv_sb = consts.tile([P, DG], F32, tag="bv")
        msk_sb = consts.tile([P, 3 * P], BF16, tag="msk")
        idb_sb = consts.tile([P, P], BF16, tag="idb")
        eps_sb = consts.tile([P, 1], F32, tag="eps")

        nc.sync.dma_start(out=idb_sb, in_=idb)
        nc.sync.dma_start(out=msk_sb, in_=msk)
        nc.vector.memset(eps_sb, LN_EPS)

        # h^T (LayerNormed x, feature-major), Q^T/K^T (feature-major),
        # V (token-major, [V | 1] per head), O^T (attn out, feature-major)
        ht_sb = big.tile([P, KC, T], BF16, tag="ht")
        qt_sb = big.tile([P, DG // P, T], BF16, tag="qt")
        kt_sb = big.tile([P, DG // P, T], BF16, tag="kt")
        v_sb = big.tile([P, NT, HG, DK + 1], BF16, tag="v")
        ot_sb = big.tile([P, DG // P, T], BF16, tag="ot")

        # ones column of [V | 1]
        nc.vector.memset(v_sb[:, :, :, DK:DK + 1], 1.0)

        # ===== Phase A+B interleaved: LayerNorm + transpose + Q/K/V =====
        # A is DVE/ACT-bound, B is PE-bound; interleaving the emission lets
        # the projection matmuls of token chunk nt backfill the PE while
        # LayerNorm of later tiles runs on the other engines.
        with (
            tc.tile_pool(name="ln", bufs=3) as ln,
            tc.tile_pool(name="lnst", bufs=4) as lnst,
            tc.tile_pool(name="abp", bufs=2, space="PSUM") as abp,
            tc.tile_pool(name="tpp", bufs=3, space="PSUM") as tpp,
        ):
            NQ = 512
            for tb in range(NT):
                xt = ln.tile([P, D], F32, tag="xt")
                nc.sync.dma_start(out=xt, in_=x[tb * P:(tb + 1) * P, :])
                if tb == 0:
                    # weights are first needed by the V projection below;
                    # issuing them after x(0) keeps the LN pipe from
                    # waiting on the strided weight loads.
                    nc.sync.dma_start(
                        out=wq_sb, in_=wq.rearrange("(c p) n -> p c n", p=P))
                    nc.sync.dma_start(
                        out=wk_sb, in_=wk.rearrange("(c p) n -> p c n", p=P))
                    nc.sync.dma_start(
                        out=wv_sb, in_=wv.rearrange("(c p) n -> p c n", p=P))
                    nc.sync.dma_start(
                        out=wo_sb, in_=wo.rearrange("(c p) n -> p c n", p=P))
                    nc.sync.dma_start(out=bq_sb, in_=bq)
                    nc.sync.dma_start(out=bk_sb, in_=bk)
                    nc.sync.dma_start(out=bv_sb, in_=bv)

                stats = lnst.tile([P, 2, 6], F32, tag="stats")
                xg = xt.rearrange("p (g d) -> p g d", g=2)
                nc.vector.bn_stats(out=stats[:, 0, :], in_=xg[:, 0, :])
                nc.vector.bn_stats(out=stats[:, 1, :], in_=xg[:, 1, :])
                mv = lnst.tile([P, 2], F32, tag="mv")
                nc.vector.bn_aggr(out=mv, in_=stats)

                rstd = lnst.tile([P, 1], F32, tag="rstd")
                nc.scalar.activation(
                    out=rstd, in_=mv[:, 1:2],
                    func=mybir.ActivationFunctionType.Sqrt,
                    bias=eps_sb, scale=1.0,
                )
                nc.vector.reciprocal(out=rstd, in_=rstd)
                nmr = lnst.tile([P, 1], F32, tag="nmr")
                nc.vector.scalar_tensor_tensor(
                    nmr, mv[:, 0:1], -1.0, rstd,
                    mybir.AluOpType.mult, mybir.AluOpType.mult)

                hn = ln.tile([P, D], BF16, tag="hn")
                nc.gpsimd.tensor_scalar(
                    hn, xt, rstd, nmr,
                    mybir.AluOpType.mult, mybir.AluOpType.add)
                for half in range(2):
                    tp = tpp.tile([P, 4 * P], BF16, tag="tp")
                    for c4 in range(4):
                        c = half * 4 + c4
                        nc.tensor.transpose(
                            tp[:, c4 * P:(c4 + 1) * P],
                            hn[:, c * P:(c + 1) * P], idb_sb)
                    dst = ht_sb[:, half * 4:(half + 1) * 4,
                                tb * P:(tb + 1) * P]
                    nc.scalar.copy(dst, tp)

                # V projection for this tile
                psv = abp.tile([P, DG], F32, tag="psv")
                for kc in range(KC):
                    nc.tensor.matmul(
                        psv,
                        ht_sb[:, kc, tb * P:(tb + 1) * P],
                        wv_sb[:, kc, :],
                        start=(kc == 0), stop=(kc == KC - 1),
                    )
                nc.vector.tensor_add(
                    v_sb[:, tb, :, 0:DK],
                    psv.rearrange("p (h d) -> p h d", h=HG), bv_sb)

                # Q/K projections for the completed 512-token chunk
                if tb % 4 == 3:
                    nt = tb // 4
                    tsl = slice(nt * NQ, (nt + 1) * NQ)
                    for oc in range(DG // P):
                        for w_sb, dst, b_sb, on_act in (
                                (wq_sb, qt_sb, bq_sb, True),
                                (wk_sb, kt_sb, bk_sb, False)):
                            ps = abp.tile([P, NQ], F32, tag="ps")
                            for kc in range(KC):
                                nc.tensor.matmul(
                                    ps,
                                    w_sb[:, kc, oc * P:(oc + 1) * P],
                                    ht_sb[:, kc, tsl],
                                    start=(kc == 0), stop=(kc == KC - 1),
                                )
                            if on_act:
                                nc.scalar.activation(
                                    out=dst[:, oc, tsl], in_=ps,
                                    func=mybir.ActivationFunctionType.Identity,
                                    bias=b_sb[:, oc:oc + 1], scale=1.0,
                                )
                            else:
                                nc.vector.tensor_scalar_add(
                                    dst[:, oc, tsl], ps, b_sb[:, oc:oc + 1])

        # ================= Phase C: banded attention (key-major) ==========
        with (
            tc.tile_pool(name="sp", bufs=2, space="PSUM") as sp,
            tc.tile_pool(name="avp", bufs=4, space="PSUM") as avp,
            tc.tile_pool(name="otp", bufs=2, space="PSUM") as otp,
            tc.tile_pool(name="smx", bufs=3) as smx,
            tc.tile_pool(name="smst", bufs=6) as smst,
        ):
            for oc in range(DG // P):           # head pair
                for hh in range(2):             # head within pair
                    p0 = hh * DK
                    h = oc * 2 + hh
                    avs = {}
                    pts = {}
                    otps_box = [None]

                    def scores(kb):
                        # s_t[k, q-window]; exp lands P^T in SBUF; band
                        # mask (diag keeps k<=q, oldest block keeps k>q)
                        # is one binary multiply against [mka | 1 | mkb]
                        njb = min(3, NT - kb)
                        jw = njb * P
                        st = sp.tile([P, 3 * P], F32, tag="st",
                                     name=f"st_{h}_{kb}")
                        nc.tensor.matmul(
                            st[:, :jw],
                            kt_sb[p0:p0 + DK, oc, kb * P:(kb + 1) * P],
                            qt_sb[p0:p0 + DK, oc, kb * P:kb * P + jw],
                            start=True, stop=True,
                        )
                        pt = smx.tile([P, 3 * P], BF16, tag="pt",
                                      name=f"pt_{h}_{kb}")
                        nc.scalar.activation(
                            out=pt[:, :jw], in_=st[:, :jw],
                            func=mybir.ActivationFunctionType.Exp,
                        )
                        # the middle block of the window is fully valid --
                        # only the diag (cols 0:P) and oldest (cols 2P:3P)
                        # blocks need masking; alternate engines by kb
                        if kb % 2 == 0:
                            nc.vector.tensor_mul(
                                pt[:, 0:P], pt[:, 0:P], msk_sb[:, 0:P])
                            if njb == 3:
                                nc.vector.tensor_mul(
                                    pt[:, 2 * P:3 * P], pt[:, 2 * P:3 * P],
                                    msk_sb[:, 2 * P:3 * P])
                        else:
                            if njb == 3:
                                nc.gpsimd.tensor_mul(
                                    pt[:, :jw], pt[:, :jw], msk_sb[:, :jw])
                            else:
                                nc.gpsimd.tensor_mul(
                                    pt[:, 0:P], pt[:, 0:P], msk_sb[:, 0:P])
                        pts[kb] = pt

                    def consume(kb):
                        # AV: token-major, denominator in column DK
                        njb = min(3, NT - kb)
                        pt = pts.pop(kb)
                        for qoff in range(njb):
                            qb = kb + qoff
                            if qb not in avs:
                                avs[qb] = avp.tile([P, DK + 1], F32,
                                                   tag="av",
                                                   name=f"av_{h}_{qb}")
                            nc.tensor.matmul(
                                avs[qb],
                                pt[:, qoff * P:(qoff + 1) * P],
                                v_sb[:, kb, h, :],
                                start=(kb == max(qb - 2, 0)),
                                stop=(kb == qb),
                            )
                        # query block kb is now complete: normalize
                        av = avs.pop(kb)
                        rden = smst.tile([P, 1], F32, tag="rden")
                        nc.vector.reciprocal(out=rden, in_=av[:, DK:DK + 1])
                        o_tok = smx.tile([P, DK], BF16, tag="otok")
                        nc.vector.tensor_scalar_mul(o_tok, av[:, 0:DK], rden)
                        # transpose back to feature-major, 4 blocks a batch
                        nonlocal_otps = otps_box
                        if kb % 4 == 0:
                            nonlocal_otps[0] = otp.tile(
                                [DK, 4 * P], BF16, tag="otb",
                                name=f"otb_{h}_{kb}")
                        nc.tensor.transpose(
                            nonlocal_otps[0][:, (kb % 4) * P:(kb % 4 + 1) * P],
                            o_tok, idb_sb)
                        if kb % 4 == 3:
                            g = kb // 4
                            dst = ot_sb[p0:p0 + DK, oc,
                                        g * 4 * P:(g + 1) * 4 * P]
                            if h < 3:
                                nc.vector.tensor_copy(dst, nonlocal_otps[0])
                            else:
                                nc.scalar.copy(dst, nonlocal_otps[0])

                    # 1-deep software pipeline: scores(kb+1) is emitted
                    # before the AV of kb so the PE always has an
                    # independent matmul ready while exp/mask complete.
                    scores(0)
                    for kb in range(NT):
                        if kb + 1 < NT:
                            scores(kb + 1)
                        consume(kb)

        # ================= Phase D: partial out-projection =================
        with (
            tc.tile_pool(name="fpp", bufs=3, space="PSUM") as fpp,
            tc.tile_pool(name="fout", bufs=4) as fout,
        ):
            NO = 512
            for tb in range(NT):
                for on in range(D // NO):
                    ps = fpp.tile([P, NO], F32, tag="ps")
                    for kd in range(DG // P):
                        nc.tensor.matmul(
                            ps,
                            ot_sb[:, kd, tb * P:(tb + 1) * P],
                            wo_sb[:, kd, on * NO:(on + 1) * NO],
                            start=(kd == 0), stop=(kd == DG // P - 1),
                        )
                    ob = fout.tile([P, NO], BF16, tag="ob")
                    if on == 0:
                        nc.vector.tensor_copy(ob, ps)
                    else:
                        nc.scalar.copy(ob, ps)
                    nc.sync.dma_start(
                        out=partial[tb * P:(tb + 1) * P, on * NO:(on + 1) * NO],
                        in_=ob)


def build_nc():
    nc = bacc.Bacc("TRN2", target_bir_lowering=False, debug=False,
                   num_devices=8)
    with tile.TileContext(nc) as tc:
        _body(tc)
    nc.compile()
    return nc


def _prep_core_inputs(x, Wq, Wk, Wv, Wo, gamma, beta):
    """Host-side prep: per-(batch, head-group) input dicts."""
    import ml_dtypes
    BF = ml_dtypes.bfloat16
    B = x.shape[0]
    ii = np.arange(P)[:, None]   # key index within block (rows)
    jj = np.arange(P)[None, :]   # query index within block (cols)
    # diag block: keep k <= q ; middle: all ; oldest block: keep k > q
    msk_np = np.concatenate(
        [(ii <= jj), np.ones((P, P), dtype=bool), (ii > jj)],
        axis=1).astype(BF)
    idb_np = np.eye(P, dtype=np.float32).astype(BF)

    in_maps = []
    for b in range(B):
        for g in range(4):
            sl = slice(g * DG, (g + 1) * DG)
            sq = np.float32(1.0 / np.sqrt(DK))
            wq_g = (gamma[:, None] * Wq[:, sl] * sq).astype(BF)
            wk_g = (gamma[:, None] * Wk[:, sl]).astype(BF)
            wv_g = (gamma[:, None] * Wv[:, sl]).astype(BF)
            bq_g = ((beta @ Wq[:, sl]) * sq).astype(np.float32)
            bk_g = (beta @ Wk[:, sl]).astype(np.float32)
            bv_g = (beta @ Wv[:, sl]).astype(np.float32)
            in_maps.append({
                "x": np.ascontiguousarray(x[b]).astype(np.float32),
                "wq": wq_g, "wk": wk_g, "wv": wv_g,
                "wo": np.ascontiguousarray(Wo[sl, :]).astype(BF),
                "bq": np.ascontiguousarray(bq_g.reshape(DG // P, P).T),
                "bk": np.ascontiguousarray(bk_g.reshape(DG // P, P).T),
                "bv": np.tile(bv_g[None, :], (P, 1)),
                "msk": msk_np, "idb": idb_np,
            })
    return in_maps


def _ntff_hook(so_path="/opt/axon/libaxon_pjrt.so"):
    import contextlib
    import ctypes

    lib = ctypes.CDLL(so_path)
    lib.axon_start_nrt_profile.argtypes = [
        ctypes.POINTER(ctypes.c_int64), ctypes.c_size_t]
    lib.axon_start_nrt_profile.restype = ctypes.c_int64
    lib.axon_stop_nrt_profile.argtypes = [ctypes.c_char_p]
    lib.axon_stop_nrt_profile.restype = ctypes.c_int64

    @contextlib.contextmanager
    def _hook(output_dir, device_ids):
        import jax
        jax.devices()
        if device_ids:
            ids = (ctypes.c_int64 * len(device_ids))(*device_ids)
            rc = lib.axon_start_nrt_profile(ids, len(device_ids))
        else:
            rc = lib.axon_start_nrt_profile(None, 0)
        if rc != 0:
            raise RuntimeError(f"axon_start_nrt_profile rc={rc}")
        try:
            yield
        finally:
            n = lib.axon_stop_nrt_profile(str(output_dir).encode())
            print(f"profile: {n} file(s) written to {output_dir}")

    return _hook


def _run_traced(nc, in_maps, trace_dir=None):
    """Execute via PJRT with NTFF capture; return BassKernelResults with
    exec_time_ns and a perfetto trace."""
    import glob
    import tempfile

    import gauge.profiler
    from concourse import bass2jax, bass_utils
    from concourse._compat import FishPath

    neff_dir = trace_dir or tempfile.mkdtemp(prefix="trn_trace_")
    hook = _ntff_hook()
    with hook(neff_dir, [0]):
        results = bass2jax.run_bass_via_pjrt(nc, in_maps, n_cores=len(in_maps))

    ntffs = glob.glob(os.path.join(neff_dir, "*_body*.ntff"))
    if not ntffs:
        print(f"no ntffs in {neff_dir}: {os.listdir(neff_dir)}")
        return bass_utils.BassKernelResults(
            results=results, instructions_and_trace=None,
            profile_json=None, exec_time_ns=None)

    profile = gauge.profiler.Profile(
        profile_path=FishPath(neff_dir),
        kernel_dev_mode=True,
        profile_on_exit=False,
        bass_kernel=nc.m,
        offline_processing=True,
        fname="*_body*",
        metadata={},
    )
    return bass_utils._process_ntff_profile(
        profile, neff_dir, nc, list(range(len(in_maps))),
        None, False, {}, trace_events=False,
    ).as_bass_kernel_results(results)


def kernel(x, Wq, Wk, Wv, Wo, bo, gamma, beta, trace=False):
    global LAST_PROFILE
    x = np.asarray(x, dtype=np.float32)
    Wq, Wk, Wv, Wo = (np.asarray(a, dtype=np.float32) for a in (Wq, Wk, Wv, Wo))
    bo = np.asarray(bo, dtype=np.float32)
    gamma = np.asarray(gamma, dtype=np.float32)
    beta = np.asarray(beta, dtype=np.float32)

    nc = build_nc()
    in_maps = _prep_core_inputs(x, Wq, Wk, Wv, Wo, gamma, beta)
    if trace:
        res = _run_traced(nc, in_maps)
    else:
        res = run_bass_kernel_spmd(nc, in_maps, core_ids=list(range(8)))
    LAST_PROFILE = {"exec_time_ns": res.exec_time_ns}

    B = x.shape[0]
    out = np.empty_like(x)
    for b in range(B):
        acc = x[b] + bo[None, :]
        for g in range(4):
            acc = acc + res.results[b * 4 + g]["partial"].astype(np.float32)
        out[b] = acc
    return out


# revision 32
# speedup vs baseline: 2.0908x; 1.0854x over previous
"""Local causal (sliding-window) attention block on 8 TRN2 NeuronCores.

Reference computation (per batch b):
    h = LayerNorm(x) * gamma + beta
    Q = h@Wq, K = h@Wk, V = h@Wv          (heads: 16 x 64)
    S = QK^T/sqrt(dk) masked to causal band of width 256
    out = x + softmax(S)@V @ Wo + bo

Sharding: 8 cores = 2 batches x 4 head-groups (4 heads each).
Each core computes LN(x_b), its head-group's Q/K/V, banded attention,
and a partial out-projection  attn_g @ Wo[g]  (token-major, [T, D]).
Host reduces: out[b] = x[b] + sum_g partial[b,g] + bo.

Attention is computed key-major: for key block kb, scores
s_t[k, q] = K_kb^T Q over the query window [kb, kb+2]; exp lands P^T
directly in SBUF (no per-block P transposes), band masking is a binary
multiply on GpSimd, and the AV matmul (lhsT = P^T slice, rhs = [V | 1])
produces token-major attention output with the softmax denominator as
column 64 -- so normalization is a per-partition scalar multiply.

gamma (and 1/sqrt(dk) for Q) are folded into the projection weights on
the host; beta enters via folded bias rows beta@W.  All matmul operands
are bf16 (PSUM accumulation in fp32).
"""

import os

import numpy as np

import concourse.bass as bass
import concourse.tile as tile
from concourse import bacc, mybir
from concourse.bass_utils import run_bass_kernel_spmd

F32 = mybir.dt.float32
BF16 = mybir.dt.bfloat16

T = 2048          # tokens per batch
D = 1024          # model dim
HG = 4            # heads per core
DK = 64           # head dim
DG = HG * DK      # head-group feature width (256)
WIN = 256         # attention window
P = 128           # partitions
NT = T // P       # 16 token tiles
KC = D // P       # 8 feature chunks
LN_EPS = 1e-5

# filled by test.py via run(trace=True)
LAST_PROFILE = {}


def _body(tc):
    nc = tc.nc

    x = nc.dram_tensor("x", [T, D], BF16, kind="ExternalInput").ap()
    wq = nc.dram_tensor("wq", [D, DG], BF16, kind="ExternalInput").ap()
    wk = nc.dram_tensor("wk", [D, DG], BF16, kind="ExternalInput").ap()
    wv = nc.dram_tensor("wv", [D, DG], BF16, kind="ExternalInput").ap()
    wo = nc.dram_tensor("wo", [DG, D], BF16, kind="ExternalInput").ap()
    bq = nc.dram_tensor("bq", [P, DG // P], F32, kind="ExternalInput").ap()
    msk2 = nc.dram_tensor("msk2", [P, 4 * P], BF16, kind="ExternalInput").ap()
    idb = nc.dram_tensor("idb", [P, P], BF16, kind="ExternalInput").ap()
    partial = nc.dram_tensor("partial", [T, D], BF16, kind="ExternalOutput").ap()

    with (
        tc.tile_pool(name="consts", bufs=1) as consts,
        tc.tile_pool(name="big", bufs=1) as big,
    ):
        # ---- resident SBUF tensors ----
        wq_sb = consts.tile([P, KC, DG], BF16, tag="wq")
        wk_sb = consts.tile([P, KC, DG], BF16, tag="wk")
        wv_sb = consts.tile([P, KC, DG], BF16, tag="wv")
        wo_sb = consts.tile([P, DG // P, D], BF16, tag="wo")
        bq_sb = consts.tile([P, DG // P], F32, tag="bq")
        msk2_sb = consts.tile([P, 4, P], BF16, tag="msk2")
        idb_sb = consts.tile([P, P], BF16, tag="idb")
        eps_sb = consts.tile([P, 1], F32, tag="eps")

        nc.sync.dma_start(out=idb_sb, in_=idb)
        nc.sync.dma_start(out=msk2_sb, in_=msk2.rearrange("p (b c) -> p b c", c=P))
        nc.vector.memset(eps_sb, LN_EPS)

        # h^T (LayerNormed x, feature-major), Q^T/K^T (feature-major),
        # V (token-major, [V | 1] per head), O^T (attn out, feature-major)
        ht_sb = big.tile([P, KC, T], BF16, tag="ht")
        qt_sb = big.tile([P, DG // P, T], BF16, tag="qt")
        kt_sb = big.tile([P, DG // P, T], BF16, tag="kt")
        v_sb = big.tile([P, NT, HG, DK + 1], BF16, tag="v")
        ot_sb = big.tile([P, DG // P, T], BF16, tag="ot")

        # ones column of [V | 1]
        nc.vector.memset(v_sb[:, :, :, DK:DK + 1], 1.0)

        # ===== Phase A+B interleaved: LayerNorm + transpose + Q/K/V =====
        # A is DVE/ACT-bound, B is PE-bound; interleaving the emission lets
        # the projection matmuls of token chunk nt backfill the PE while
        # LayerNorm of later tiles runs on the other engines.
        with (
            tc.tile_pool(name="ln", bufs=6) as ln,
            tc.tile_pool(name="lnst", bufs=9) as lnst,
            tc.tile_pool(name="abp", bufs=2, space="PSUM") as abp,
            tc.tile_pool(name="tpp", bufs=3, space="PSUM") as tpp,
        ):
            NQ = 512
            for tb in range(NT):
                xt = ln.tile([P, D], BF16, tag="xt")
                nc.sync.dma_start(out=xt, in_=x[tb * P:(tb + 1) * P, :])
                if tb == 0:
                    # weights are first needed by the V projection below;
                    # issuing them after x(0) keeps the LN pipe from
                    # waiting on the strided weight loads.
                    nc.sync.dma_start(
                        out=wq_sb, in_=wq.rearrange("(c p) n -> p c n", p=P))
                    nc.sync.dma_start(
                        out=wk_sb, in_=wk.rearrange("(c p) n -> p c n", p=P))
                    nc.sync.dma_start(
                        out=wv_sb, in_=wv.rearrange("(c p) n -> p c n", p=P))
                    nc.sync.dma_start(
                        out=wo_sb, in_=wo.rearrange("(c p) n -> p c n", p=P))
                    nc.sync.dma_start(out=bq_sb, in_=bq)

                stats = lnst.tile([P, 2, 6], F32, tag="stats")
                xg = xt.rearrange("p (g d) -> p g d", g=2)
                nc.vector.bn_stats(out=stats[:, 0, :], in_=xg[:, 0, :])
                nc.vector.bn_stats(out=stats[:, 1, :], in_=xg[:, 1, :])
                mv = lnst.tile([P, 2], F32, tag="mv")
                nc.vector.bn_aggr(out=mv, in_=stats)

                rstd = lnst.tile([P, 1], F32, tag="rstd")
                nc.scalar.activation(
                    out=rstd, in_=mv[:, 1:2],
                    func=mybir.ActivationFunctionType.Sqrt,
                    bias=eps_sb, scale=1.0,
                )
                nc.vector.reciprocal(out=rstd, in_=rstd)
                nmr = lnst.tile([P, 1], F32, tag="nmr")
                nc.vector.scalar_tensor_tensor(
                    nmr, mv[:, 0:1], -1.0, rstd,
                    mybir.AluOpType.mult, mybir.AluOpType.mult)

                hn = ln.tile([P, D], BF16, tag="hn")
                nc.gpsimd.tensor_scalar(
                    hn, xt, rstd, nmr,
                    mybir.AluOpType.mult, mybir.AluOpType.add)
                for half in range(2):
                    tp = tpp.tile([P, 4 * P], BF16, tag="tp")
                    for c4 in range(4):
                        c = half * 4 + c4
                        nc.tensor.transpose(
                            tp[:, c4 * P:(c4 + 1) * P],
                            hn[:, c * P:(c + 1) * P], idb_sb)
                    dst = ht_sb[:, half * 4:(half + 1) * 4,
                                tb * P:(tb + 1) * P]
                    nc.scalar.copy(dst, tp)

                # V projection for this tile
                psv = abp.tile([P, DG], F32, tag="psv")
                for kc in range(KC):
                    nc.tensor.matmul(
                        psv,
                        ht_sb[:, kc, tb * P:(tb + 1) * P],
                        wv_sb[:, kc, :],
                        start=(kc == 0), stop=(kc == KC - 1),
                    )
                # bv is not applied on-device: softmax weights sum to 1,
                # so the V bias adds a constant bv to every attention
                # output; bv @ Wo is folded into the host-side bo add.
                vdst = v_sb[:, tb, :, 0:DK]
                vsrc = psv.rearrange("p (h d) -> p h d", h=HG)
                if tb % 2 == 0:
                    nc.vector.tensor_copy(vdst, vsrc)
                else:
                    nc.scalar.copy(vdst, vsrc)

                # Q/K projections for the completed 512-token chunk
                if tb % 4 == 3:
                    nt = tb // 4
                    tsl = slice(nt * NQ, (nt + 1) * NQ)
                    for oc in range(DG // P):
                        # bk is not applied: a K bias shifts all scores of
                        # a query uniformly, which softmax cancels.
                        for w_sb, dst, on_act in (
                                (wq_sb, qt_sb, True),
                                (wk_sb, kt_sb, False)):
                            ps = abp.tile([P, NQ], F32, tag="ps")
                            for kc in range(KC):
                                nc.tensor.matmul(
                                    ps,
                                    w_sb[:, kc, oc * P:(oc + 1) * P],
                                    ht_sb[:, kc, tsl],
                                    start=(kc == 0), stop=(kc == KC - 1),
                                )
                            if on_act:
                                nc.scalar.activation(
                                    out=dst[:, oc, tsl], in_=ps,
                                    func=mybir.ActivationFunctionType.Identity,
                                    bias=bq_sb[:, oc:oc + 1], scale=1.0,
                                )
                            else:
                                nc.vector.tensor_copy(dst[:, oc, tsl], ps)

        # ================= Phase C: banded attention (key-major) ==========
        # Both heads of an oc chunk share one chain per key block: one
        # bf16 psum scores tile, one exp, strided masks over both heads,
        # one [V|1] psum tile, one transpose per query block.
        with (
            tc.tile_pool(name="sp", bufs=2, space="PSUM") as sp,
            tc.tile_pool(name="avp", bufs=3, space="PSUM") as avp,
            tc.tile_pool(name="otp", bufs=1, space="PSUM") as otp,
            tc.tile_pool(name="smx", bufs=3) as smx,
            tc.tile_pool(name="smst", bufs=6) as smst,
        ):
            SW = 4 * P   # per-head stride pads each head to one psum bank
            for oc in range(DG // P):           # head pair
                avs = {}
                pts = {}
                otps_box = [None]

                def scores(kb):
                    # s_t[k, q-window] for both heads; exp lands P^T in
                    # SBUF; band mask (diag keeps k<=q, oldest keeps k>q)
                    njb = min(3, NT - kb)
                    jw = njb * P
                    st = sp.tile([P, 2, SW], F32, tag="st",
                                 name=f"st_{oc}_{kb}")
                    for hh in range(2):
                        p0 = hh * DK
                        nc.tensor.matmul(
                            st[:, hh, :jw],
                            kt_sb[p0:p0 + DK, oc, kb * P:(kb + 1) * P],
                            qt_sb[p0:p0 + DK, oc, kb * P:kb * P + jw],
                            start=True, stop=True,
                        )
                    pt = smx.tile([P, 2, 3 * P], BF16, tag="pt",
                                  name=f"pt_{oc}_{kb}")
                    nc.scalar.activation(
                        out=pt[:, :, :jw], in_=st[:, :, :jw],
                        func=mybir.ActivationFunctionType.Exp,
                    )
                    if kb % 2 == 0:
                        nc.vector.tensor_mul(
                            pt[:, :, 0:P], pt[:, :, 0:P], msk2_sb[:, 0:2, :])
                        if njb == 3:
                            nc.vector.tensor_mul(
                                pt[:, :, 2 * P:3 * P], pt[:, :, 2 * P:3 * P],
                                msk2_sb[:, 2:4, :])
                    else:
                        nc.gpsimd.tensor_mul(
                            pt[:, :, 0:P], pt[:, :, 0:P], msk2_sb[:, 0:2, :])
                        if njb == 3:
                            nc.gpsimd.tensor_mul(
                                pt[:, :, 2 * P:3 * P], pt[:, :, 2 * P:3 * P],
                                msk2_sb[:, 2:4, :])
                    pts[kb] = pt

                def consume(kb):
                    # AV: token-major, denominator in column DK, both
                    # heads side by side in one psum tile
                    njb = min(3, NT - kb)
                    pt = pts.pop(kb)
                    for qoff in range(njb):
                        qb = kb + qoff
                        if qb not in avs:
                            avs[qb] = avp.tile([P, 2, DK + 1], F32,
                                               tag="av",
                                               name=f"av_{oc}_{qb}")
                        for hh in range(2):
                            nc.tensor.matmul(
                                avs[qb][:, hh, :],
                                pt[:, hh, qoff * P:(qoff + 1) * P],
                                v_sb[:, kb, oc * 2 + hh, :],
                                start=(kb == max(qb - 2, 0)),
                                stop=(kb == qb),
                            )
                    # query block kb is now complete: normalize
                    av = avs.pop(kb)
                    rden = smst.tile([P, 2], F32, tag="rden")
                    nc.vector.reciprocal(out=rden, in_=av[:, :, DK:DK + 1])
                    o_tok = smx.tile([P, 2, DK], BF16, tag="otok")
                    for hh in range(2):
                        nc.vector.tensor_scalar_mul(
                            o_tok[:, hh, :], av[:, hh, 0:DK],
                            rden[:, hh:hh + 1])
                    # transpose back to feature-major, 4 blocks a batch
                    if kb % 4 == 0:
                        otps_box[0] = otp.tile(
                            [P, 4 * P], BF16, tag="otb",
                            name=f"otb_{oc}_{kb}")
                    nc.tensor.transpose(
                        otps_box[0][:, (kb % 4) * P:(kb % 4 + 1) * P],
                        o_tok, idb_sb)
                    if kb % 4 == 3:
                        g = kb // 4
                        dst = ot_sb[:, oc, g * 4 * P:(g + 1) * 4 * P]
                        if (oc + kb // 4) % 3 == 0:
                            nc.scalar.copy(dst, otps_box[0])
                        else:
                            nc.vector.tensor_copy(dst, otps_box[0])

                # software pipeline: scores(kb+1) is emitted before the
                # AV of kb so the PE always has independent matmuls
                # ready while exp/mask complete.
                scores(0)
                for kb in range(NT):
                    if kb + 1 < NT:
                        scores(kb + 1)
                    consume(kb)

        # ================= Phase D: partial out-projection =================
        with (
            tc.tile_pool(name="fpp", bufs=3, space="PSUM") as fpp,
            tc.tile_pool(name="fout", bufs=4) as fout,
        ):
            NO = 512
            for tb in range(NT):
                for on in range(D // NO):
                    ps = fpp.tile([P, NO], F32, tag="ps")
                    for kd in range(DG // P):
                        nc.tensor.matmul(
                            ps,
                            ot_sb[:, kd, tb * P:(tb + 1) * P],
                            wo_sb[:, kd, on * NO:(on + 1) * NO],
                            start=(kd == 0), stop=(kd == DG // P - 1),
                        )
                    ob = fout.tile([P, NO], BF16, tag="ob")
                    if on == 0:
                        nc.vector.tensor_copy(ob, ps)
                    else:
                        nc.scalar.copy(ob, ps)
                    nc.sync.dma_start(
                        out=partial[tb * P:(tb + 1) * P, on * NO:(on + 1) * NO],
                        in_=ob)


def build_nc():
    nc = bacc.Bacc("TRN2", target_bir_lowering=False, debug=False,
                   num_devices=8)
    with tile.TileContext(nc) as tc:
        _body(tc)
    nc.compile()
    return nc


def _prep_core_inputs(x, Wq, Wk, Wv, Wo, gamma, beta):
    """Host-side prep: per-(batch, head-group) input dicts."""
    import ml_dtypes
    BF = ml_dtypes.bfloat16
    B = x.shape[0]
    ii = np.arange(P)[:, None]   # key index within block (rows)
    jj = np.arange(P)[None, :]   # query index within block (cols)
    # diag block: keep k <= q ; oldest block: keep k > q.  Each mask is
    # duplicated so one strided multiply covers both heads of a pair.
    mka = (ii <= jj)
    mkb = (ii > jj)
    msk2_np = np.concatenate([mka, mka, mkb, mkb], axis=1).astype(BF)
    idb_np = np.eye(P, dtype=np.float32).astype(BF)

    in_maps = []
    for b in range(B):
        for g in range(4):
            sl = slice(g * DG, (g + 1) * DG)
            sq = np.float32(1.0 / np.sqrt(DK))
            wq_g = (gamma[:, None] * Wq[:, sl] * sq).astype(BF)
            wk_g = (gamma[:, None] * Wk[:, sl]).astype(BF)
            wv_g = (gamma[:, None] * Wv[:, sl]).astype(BF)
            bq_g = ((beta @ Wq[:, sl]) * sq).astype(np.float32)
            in_maps.append({
                "x": np.ascontiguousarray(x[b]).astype(BF),
                "wq": wq_g, "wk": wk_g, "wv": wv_g,
                "wo": np.ascontiguousarray(Wo[sl, :]).astype(BF),
                "bq": np.ascontiguousarray(bq_g.reshape(DG // P, P).T),
                "msk2": msk2_np, "idb": idb_np,
            })
    return in_maps


def _ntff_hook(so_path="/opt/axon/libaxon_pjrt.so"):
    import contextlib
    import ctypes

    lib = ctypes.CDLL(so_path)
    lib.axon_start_nrt_profile.argtypes = [
        ctypes.POINTER(ctypes.c_int64), ctypes.c_size_t]
    lib.axon_start_nrt_profile.restype = ctypes.c_int64
    lib.axon_stop_nrt_profile.argtypes = [ctypes.c_char_p]
    lib.axon_stop_nrt_profile.restype = ctypes.c_int64

    @contextlib.contextmanager
    def _hook(output_dir, device_ids):
        import jax
        jax.devices()
        if device_ids:
            ids = (ctypes.c_int64 * len(device_ids))(*device_ids)
            rc = lib.axon_start_nrt_profile(ids, len(device_ids))
        else:
            rc = lib.axon_start_nrt_profile(None, 0)
        if rc != 0:
            raise RuntimeError(f"axon_start_nrt_profile rc={rc}")
        try:
            yield
        finally:
            n = lib.axon_stop_nrt_profile(str(output_dir).encode())
            print(f"profile: {n} file(s) written to {output_dir}")

    return _hook


def _run_traced(nc, in_maps, trace_dir=None):
    """Execute via PJRT with NTFF capture; return BassKernelResults with
    exec_time_ns and a perfetto trace."""
    import glob
    import tempfile

    import gauge.profiler
    from concourse import bass2jax, bass_utils
    from concourse._compat import FishPath

    neff_dir = trace_dir or tempfile.mkdtemp(prefix="trn_trace_")
    hook = _ntff_hook()
    with hook(neff_dir, [0]):
        results = bass2jax.run_bass_via_pjrt(nc, in_maps, n_cores=len(in_maps))

    ntffs = glob.glob(os.path.join(neff_dir, "*_body*.ntff"))
    if not ntffs:
        print(f"no ntffs in {neff_dir}: {os.listdir(neff_dir)}")
        return bass_utils.BassKernelResults(
            results=results, instructions_and_trace=None,
            profile_json=None, exec_time_ns=None)

    profile = gauge.profiler.Profile(
        profile_path=FishPath(neff_dir),
        kernel_dev_mode=True,
        profile_on_exit=False,
        bass_kernel=nc.m,
        offline_processing=True,
        fname="*_body*",
        metadata={},
    )
    return bass_utils._process_ntff_profile(
        profile, neff_dir, nc, list(range(len(in_maps))),
        None, False, {}, trace_events=False,
    ).as_bass_kernel_results(results)


def kernel(x, Wq, Wk, Wv, Wo, bo, gamma, beta, trace=False):
    global LAST_PROFILE
    x = np.asarray(x, dtype=np.float32)
    Wq, Wk, Wv, Wo = (np.asarray(a, dtype=np.float32) for a in (Wq, Wk, Wv, Wo))
    bo = np.asarray(bo, dtype=np.float32)
    gamma = np.asarray(gamma, dtype=np.float32)
    beta = np.asarray(beta, dtype=np.float32)

    nc = build_nc()
    in_maps = _prep_core_inputs(x, Wq, Wk, Wv, Wo, gamma, beta)
    if trace:
        res = _run_traced(nc, in_maps)
    else:
        res = run_bass_kernel_spmd(nc, in_maps, core_ids=list(range(8)))
    LAST_PROFILE = {"exec_time_ns": res.exec_time_ns}

    # the V bias is not applied on-device: softmax rows sum to 1, so it
    # contributes the constant (beta @ Wv) @ Wo to every token.
    bv_full = (beta @ Wv).astype(np.float32)
    const_row = bo + bv_full @ Wo

    B = x.shape[0]
    out = np.empty_like(x)
    for b in range(B):
        acc = x[b] + const_row[None, :]
        for g in range(4):
            acc = acc + res.results[b * 4 + g]["partial"].astype(np.float32)
        out[b] = acc
    return out
